# revision 19
# baseline (speedup 1.0000x reference)
"""Causal multi-head attention (B=2, S=2048, D=1024, H=16, HD=64) on 8 trn2 cores.

Sharding: 2 heads per core x both batches (head-parallel QKV/attention/out-proj,
Wo h-split => per-core partial outputs, summed on host).

bf16 datapath (X/W/Q/K/V/e/otn bf16; projection + attnV accumulation and
softmax denominators fp32; scores and out-proj PSUM bf16 since they are
single-shot) — measured l2 ~6e-3 vs fp64 on CPU, well under the 2e-2 gate.
bf16 halves SBUF so both batches' X^T stream in from kernel start, and makes
N<256 matmuls run at 1 cycle/row so diagonal chunks are width-trimmed.

The PE HAM clock-gate re-throttles to 1.2GHz when it sees idle windows (the
previous kernel ran ~half its span at 1.2GHz), so everything is structured to
keep the PE stream dense:
  - scores run one group (= one k-chunk, both heads row-tiled at partition
    bases 0/64) ahead of the exp that consumes them (2-slot PSUM ring)
  - all projections are 8-matmul packets through a small PSUM ring; batch 0's
    run at kernel start (input DMA is column-block-major so packet 0 starts
    ~1.5us in), batch 1's sit in a cost-tracked filler queue drained into
    batch 0's attention slack (exp on ACT is the per-group limiter)
  - out-proj is deferred one item per group into the following q-tile's
    attention instead of bursting at the q-tile boundary
  - causal masking is a tiny DVE triangle-multiply on the 128-wide diagonal
    band (GPSIMD affine_select was 1.4us inside the score->exp->attnV chain)
  - softmax denominators ride the attnV matmul as a 65th ones-column in V^T;
    per q-tile one K=2 matmul broadcasts both heads' reciprocals

Engine budget per core: PE ~100us (proj 41 + scores 14 + attnV 29 + outproj
14 + transposes), ACT ~105us (exp 92 = the softmax floor + proj/out copies),
DVE ~65us (other PSUM evacuation, norm, masks), GPSIMD idle.

PSUM (8 banks): scores ring 2x[128,2,512]bf16 (2), o ring 4x[65,512]f32
(attnV accum + denominators) (4), small ring 2x 1-bank (proj packets f32,
transposes, bc, out-proj pairs bf16) (2).
"""

import numpy as np

import concourse.bass as bass
import concourse.mybir as mybir
import concourse.tile as tile
from concourse import bacc
from concourse.bass_utils import run_bass_kernel_spmd
from concourse.masks import make_identity, make_upper_triangular
from concourse.dve_ops import (RECIP_APPROX_FAST_CONSTS,
                               RECIPROCAL_APPROX_FAST)

F32 = mybir.dt.float32
F32R = mybir.dt.float32r
BF16 = mybir.dt.bfloat16
AF = mybir.ActivationFunctionType

B, S, D, H, HD = 2, 2048, 1024, 16, 64
NCORES = 8
HPC = H // NCORES          # heads per core = 2
HH = HPC * HD              # 128 concat head dims per core
P = 128
DC = D // P                # 8 d-chunks
NQ = 512                   # q tile (psum bank width fp32)
QJ = S // NQ               # 4 q tiles
KC = S // P                # 16 k chunks

_NC_CACHE = {}


def _build_nc(with_bias_qkv: bool, causal: bool):
    key = (with_bias_qkv, causal)
    if key in _NC_CACHE:
        return _NC_CACHE[key]

    nc = bacc.Bacc("TRN2", target_bir_lowering=False, debug=False)
    xt = nc.dram_tensor("xt", [B, D, S], BF16, kind="ExternalInput")
    wq = nc.dram_tensor("wq", [P, DC, HH], BF16, kind="ExternalInput")
    wk = nc.dram_tensor("wk", [P, DC, HH], BF16, kind="ExternalInput")
    wv = nc.dram_tensor("wv", [P, DC, HH], BF16, kind="ExternalInput")
    wo = nc.dram_tensor("wo", [HH, D], BF16, kind="ExternalInput")
    if with_bias_qkv:
        bqkv = nc.dram_tensor("bqkv", [3, HH], F32, kind="ExternalInput")
    out = nc.dram_tensor("out", [B, S, D], BF16, kind="ExternalOutput")

    with tile.TileContext(nc) as tc:
        with (
            tc.tile_pool(name="const", bufs=1) as cpool,
            tc.tile_pool(name="xtp", bufs=1) as xt_pool,
            tc.tile_pool(name="qkv", bufs=1) as qkv_pool,
            tc.tile_pool(name="ep", bufs=3) as e_pool,
            tc.tile_pool(name="nrm", bufs=2) as nrm_pool,
            tc.tile_pool(name="osb", bufs=4) as osb_pool,
            tc.tile_pool(name="psb", bufs=2, space="PSUM") as ps_big,
            tc.tile_pool(name="pss", bufs=2, space="PSUM") as ps_sm,
            tc.tile_pool(name="pso", bufs=2, space="PSUM") as ps_o,
        ):
            # ---- weights + X^T DMA, ordered so V packet j0 can start early
            wv_sb = cpool.tile([P, DC, HH], BF16, tag="wv", name="wv_sb")
            nc.sync.dma_start(wv_sb[:], wv[:])
            if with_bias_qkv:
                bqkvt_sb = cpool.tile([HH, 3], F32, tag="bqkvt", name="bqkvt_sb")
                for i in range(3):
                    nc.sync.dma_start(
                        bqkvt_sb[:, i:i + 1], bqkv[i:i + 1, :].rearrange("a f -> f a")
                    )
            xt_sb = [None, None]
            for b in range(B):
                xt_sb[b] = xt_pool.tile([P, DC, S], BF16, tag=f"xt{b}",
                                        name=f"xt{b}_sb")

            def load_xt(b, j):
                nc.sync.dma_start(
                    xt_sb[b][:, :, j * NQ:(j + 1) * NQ],
                    xt[b, :, j * NQ:(j + 1) * NQ].rearrange(
                        "(o p) n -> p o n", p=P),
                )

            load_xt(0, 0)
            load_xt(0, 1)
            wq_sb = cpool.tile([P, DC, HH], BF16, tag="wq", name="wq_sb")
            nc.sync.dma_start(wq_sb[:], wq[:])
            wk_sb = cpool.tile([P, DC, HH], BF16, tag="wk", name="wk_sb")
            nc.sync.dma_start(wk_sb[:], wk[:])
            load_xt(0, 2)
            load_xt(0, 3)
            wo_sb = cpool.tile([P, D], BF16, tag="wo", name="wo_sb")
            nc.sync.dma_start(wo_sb[:], wo[:])
            for j in range(QJ):
                load_xt(1, j)

            ident_sb = cpool.tile([P, P], BF16, tag="ident", name="ident_sb")
            make_identity(nc, ident_sb[:])
            # mtri[p, j] = 1 if j >= p else 0: causal keep-mask for the
            # 128-wide diagonal band of a k-chunk
            mtri_sb = cpool.tile([P, P], BF16, tag="mtri", name="mtri_sb")
            make_upper_triangular(nc, mtri_sb[:], val=1.0, diag=True)
            # ones2: row 0 selects head-0 partitions, row 32 head-1 (engine
            # partition bases must be multiples of 32) -- one K=33 matmul
            # broadcasts both heads' 1/denom over their 64 partitions
            ones2_sb = cpool.tile([33, P], F32R, tag="ones2", name="ones2_sb")
            nc.vector.memset(ones2_sb[:].bitcast(F32), 0.0)
            nc.vector.memset(ones2_sb[0:1, 0:HD].bitcast(F32), 1.0)
            nc.vector.memset(ones2_sb[32:33, HD:HH].bitcast(F32), 1.0)

            # norm scratch: persistent; dead rows 1-31 kept finite once
            rsq_sb = cpool.tile([33, NQ], F32, tag="rsq", name="rsq_sb")
            nc.vector.memset(rsq_sb[:], 1.0)
            rinq_sb = cpool.tile([33, NQ], F32R, tag="rinq", name="rinq_sb")

            warm_sb = cpool.tile([P, NQ], BF16, tag="warm", name="warm_sb")
            nc.vector.memset(warm_sb[:], 0.0)

            def pe_warmup(n):
                wp = ps_sm.tile([P, NQ], F32, tag="sm", name="warm_ps")
                for i in range(n):
                    nc.tensor.matmul(wp[:], lhsT=ident_sb[:], rhs=warm_sb[:],
                                     start=(i == 0), stop=(i == n - 1))

            # per-batch state
            st = [dict() for _ in range(B)]

            def alloc_qkv(b):
                st[b]["qt"] = qkv_pool.tile([P, QJ, NQ], BF16, tag=f"qt{b}",
                                            name="qt_sb")
                st[b]["kt"] = qkv_pool.tile([P, QJ, NQ], BF16, tag=f"kt{b}",
                                            name="kt_sb")
                st[b]["vt"] = qkv_pool.tile([P, QJ, NQ], BF16, tag="vt",
                                            name="vt_sb")
                v_sb = qkv_pool.tile([P, KC, HPC, HD + 2], BF16, tag=f"v{b}",
                                     name="v_sb")
                # 65th lhsT column = 1.0 -> attnV also accumulates sum(e)
                nc.vector.memset(v_sb[:, :, :, HD:HD + 1], 1.0)
                st[b]["v"] = v_sb

            def proj_packet(b, w_idx, j, half=None):
                """One NQ block of one projection: 8 accumulating matmuls
                (fp32 psum) through the small ring + one PSUM->SBUF copy
                (ACT for batch 0 / DVE for batch 1, balancing the phases).
                half=0 runs d 0-3 and returns the psum tile; half=(1, pps)
                finishes d 4-7 + copy."""
                w_sb = (wq_sb, wk_sb, wv_sb)[w_idx]
                dst = (st[b]["qt"], st[b]["kt"], st[b]["vt"])[w_idx]
                if half is None:
                    pps = ps_sm.tile([P, NQ], F32, tag="sm", name="pkt_ps")
                    dr = range(DC)
                elif half == 0:
                    pps = ps_sm.tile([P, NQ], F32, tag="sm", name="pkt_ps")
                    dr = range(DC // 2)
                else:
                    pps = half[1]
                    dr = range(DC // 2, DC)
                for d in dr:
                    nc.tensor.matmul(
                        pps[:],
                        lhsT=w_sb[:, d, :],
                        rhs=xt_sb[b][:, d, j * NQ:(j + 1) * NQ],
                        start=(d == 0), stop=(d == DC - 1),
                    )
                if half == 0:
                    return pps
                if with_bias_qkv:
                    if b == 0:
                        nc.scalar.activation(
                            dst[:, j, :], pps[:], AF.Identity,
                            bias=bqkvt_sb[:, w_idx:w_idx + 1],
                        )
                    else:
                        nc.vector.tensor_scalar_add(
                            dst[:, j, :], pps[:], bqkvt_sb[:, w_idx:w_idx + 1]
                        )
                elif b == 0:
                    nc.scalar.copy(dst[:, j, :], pps[:])
                else:
                    nc.vector.tensor_copy(dst[:, j, :], pps[:])

            def v_transpose(b, sc):
                tp = ps_sm.tile([P, P], BF16, tag="sm", name="tr_ps")
                nc.tensor.transpose(
                    tp[:], st[b]["vt"][:, sc // 4, (sc % 4) * P:(sc % 4 + 1) * P],
                    ident_sb[:],
                )
                # both heads in one strided copy: [128,128] -> [128, 2, 64]
                nc.vector.tensor_copy(
                    st[b]["v"][:, sc, :, 0:HD],
                    tp[:].rearrange("p (h d) -> p h d", h=HPC),
                )

            # ------------- attention -------------
            def group_seq():
                for qj in (3, 2, 1, 0):
                    nk = 4 * (qj + 1) if causal else KC
                    for ki in range(nk):
                        yield qj, ki, nk

            def emit_scores(b, qj, ki):
                qt, kt = st[b]["qt"], st[b]["kt"]
                sp = ps_big.tile([P, 2, NQ], F32, tag="big", name="st_ps")
                o = ki - 4 * qj if causal else -1
                q0 = o * P if o > 0 else 0
                for h in range(HPC):
                    h0 = h * HD
                    nc.tensor.matmul(
                        sp[:, h, q0:NQ],
                        lhsT=kt[h0:h0 + HD, ki // 4, (ki % 4) * P:(ki % 4 + 1) * P],
                        rhs=qt[h0:h0 + HD, qj, q0:NQ],
                        start=True, stop=True,
                    )
                return sp

            def emit_exp(b, qj, ki, sp):
                e_sb = e_pool.tile([P, 2, NQ], BF16, tag="e", name="e_sb")
                o = ki - 4 * qj if causal else -1
                q0 = o * P if o > 0 else 0
                nc.scalar.activation(e_sb[:, :, q0:NQ], sp[:, :, q0:NQ],
                                     AF.Exp, scale=0.125)
                if causal and o >= 0:
                    # zero the masked triangle of the 128-wide diagonal band
                    for h in range(HPC):
                        nc.vector.tensor_mul(
                            e_sb[:, h, o * P:(o + 1) * P],
                            e_sb[:, h, o * P:(o + 1) * P],
                            mtri_sb[:],
                        )
                return e_sb

            def emit_attnv(b, qj, ki, nk, e_sb, o_ps):
                v = st[b]["v"]
                o = ki - 4 * qj if causal else -1
                q0 = o * P if o > 0 else 0
                for h in range(HPC):
                    nc.tensor.matmul(
                        o_ps[h][:, q0:NQ],
                        lhsT=v[:, ki, h, 0:HD + 1],
                        rhs=e_sb[:, h, q0:NQ],
                        start=(ki == 0), stop=(ki == nk - 1),
                    )

            def emit_norm(b, qj, o_ps):
                otn = st[b]["otn"]
                for h in range(HPC):
                    nc.vector.tensor_copy(rsq_sb[32 * h:32 * h + 1, :],
                                          o_ps[h][HD:HD + 1, :])
                c = RECIP_APPROX_FAST_CONSTS
                nc.vector._custom_dve(
                    RECIPROCAL_APPROX_FAST, out=rinq_sb[:], in0=rsq_sb[:],
                    s0=c["s0"], s1=c["s1"], imm2=c["imm2"],
                )
                bc_ps = ps_sm.tile([P, NQ], F32, tag="sm", name="bc_ps")
                nc.tensor.matmul(bc_ps[:], lhsT=ones2_sb[:], rhs=rinq_sb[:],
                                 start=True, stop=True)
                # DVE reads at most one PSUM operand: stage o through SBUF
                ot = nrm_pool.tile([HH, NQ], F32, tag="ot", name="ot")
                for h in range(HPC):
                    h0 = h * HD
                    nc.vector.tensor_copy(ot[h0:h0 + HD, :], o_ps[h][0:HD, :])
                for h in range(HPC):
                    h0 = h * HD
                    nc.vector.tensor_mul(
                        otn[h0:h0 + HD, qj, :], ot[h0:h0 + HD, :],
                        bc_ps[h0:h0 + HD, :]
                    )

            def outproj_item(b, qj, sc4, _ct=[0]):
                otn = st[b]["otn"]
                sc = qj * 4 + sc4
                out_sb = osb_pool.tile([P, 2, NQ], BF16, tag="out", name="out_sb")
                for fc in range(2):
                    ops = ps_sm.tile([P, NQ], F32, tag="sm", name="op_ps")
                    nc.tensor.matmul(
                        ops[:],
                        lhsT=otn[:, qj, sc4 * P:(sc4 + 1) * P],
                        rhs=wo_sb[:, fc * NQ:(fc + 1) * NQ],
                        start=True, stop=True,
                    )
                    if _ct[0] % 5 < 2:
                        nc.scalar.copy(out_sb[:, fc, :], ops[:])
                    else:
                        nc.vector.tensor_copy(out_sb[:, fc, :], ops[:])
                    _ct[0] += 1
                nc.sync.dma_start(
                    out[b, sc * P:(sc + 1) * P, :],
                    out_sb.rearrange("p a n -> p (a n)"),
                )

            def flush_fillers(fillers, pending, keep=0):
                while len(fillers) > keep or pending[0] is not None:
                    if pending[0] is not None:
                        fn, pending[0] = pending[0], None
                        fn()
                    else:
                        pending[0] = fillers.pop(0)[1]()

            def attention(b, fillers, fill_debt, opq, pending, due=()):
                st[b]["otn"] = qkv_pool.tile([P, QJ, NQ], BF16, tag=f"otn{b}",
                                             name="otn_sb")
                due = dict(due)
                seq = list(group_seq())
                sp_next = emit_scores(b, *seq[0][:2])
                o_ps = None
                for i, (qj, ki, nk) in enumerate(seq):
                    if i in due:
                        flush_fillers(fillers, pending, keep=due[i])
                    if ki == 0:
                        o_ps = [ps_o.tile([HD + 1, NQ], F32, tag="o",
                                          name=f"o_ps{h}") for h in range(HPC)]
                    sp = sp_next
                    if i + 1 < len(seq):
                        sp_next = emit_scores(b, *seq[i + 1][:2])
                    e_sb = emit_exp(b, qj, ki, sp)
                    emit_attnv(b, qj, ki, nk, e_sb, o_ps)
                    if ki == nk - 1:
                        emit_norm(b, qj, o_ps)
                        for sc4 in range(4):
                            opq.append((b, qj, sc4))
                    elif pending[0] is not None:
                        fn, pending[0] = pending[0], None
                        fn()
                    elif opq:
                        outproj_item(*opq.pop(0))
                    # exp on ACT (~0.93us avg) outpaces the group's PE work
                    # (~0.65us): drain the filler queue with the difference
                    fill_debt[0] += 420
                    while (pending[0] is None and fillers
                           and fill_debt[0] >= fillers[0][0]):
                        cost, fn = fillers.pop(0)
                        fill_debt[0] -= cost
                        pending[0] = fn()

            # ---------------- program ----------------
            pe_warmup(18)
            alloc_qkv(0)
            # batch-0 projections: V first (j 0..3) so transposes interleave,
            # Q in j 3..0 (attention consumes q-tile 3 first), K in j 0..3
            p0 = [(2, 0), (2, 1)]
            p0 += [("t", 0), ("t", 1), (2, 2), ("t", 2), ("t", 3), (2, 3)]
            p0 += [("t", 4), ("t", 5), (0, 3), ("t", 6), ("t", 7)]
            p0 += [("t", 8), ("t", 9), (1, 0), ("t", 10), ("t", 11)]
            p0 += [("t", 12), ("t", 13), ("t", 14), ("t", 15)]
            for w_idx, j in p0:
                if w_idx == "t":
                    v_transpose(0, j)
                else:
                    proj_packet(0, w_idx, j)

            alloc_qkv(1)
            # filler order: V packets feed transposes; Q packets in j 3..0
            # (b1 attention reads q-tile 3 first), K packets in j 0..3.
            # Packets split in two 4-matmul halves; half A returns half B,
            # which MUST be the next queue-driven small-ring consumer.
            def make_pkt(w, j, b=1):
                def half_a():
                    pps = proj_packet(b, w, j, half=0)
                    return lambda: proj_packet(b, w, j, half=(1, pps))
                return half_a

            fillers = [(1700, make_pkt(1, 1, b=0)),
                       (1700, make_pkt(1, 2, b=0)),
                       (1700, make_pkt(1, 3, b=0)),
                       (1700, make_pkt(0, 2, b=0)),
                       (1700, make_pkt(0, 1, b=0)),
                       (1700, make_pkt(0, 0, b=0))]
            def tr1(s):
                return (250, lambda ss=s: v_transpose(1, ss) and None)

            # batch-1 order: [Q3, V0, tr0-3, K0] unblock its first groups
            # (flushed at the bridge); then V/tr/K waves sized to the groups
            # that consume them, Q j2/j1/j0 last -- all due-date forced
            fillers.append((1700, make_pkt(0, 3)))
            fillers.append((1700, make_pkt(2, 0)))
            fillers += [tr1(0), tr1(1), tr1(2), tr1(3)]
            fillers.append((1700, make_pkt(1, 0)))
            fillers.append((1700, make_pkt(2, 1)))
            fillers += [tr1(4), tr1(5), tr1(6), tr1(7)]
            fillers.append((1700, make_pkt(1, 1)))
            fillers.append((1700, make_pkt(2, 2)))
            fillers += [tr1(8), tr1(9), tr1(10), tr1(11)]
            fillers.append((1700, make_pkt(1, 2)))
            fillers.append((1700, make_pkt(2, 3)))
            fillers += [tr1(12), tr1(13), tr1(14), tr1(15)]
            fillers.append((1700, make_pkt(1, 3)))
            fillers.append((1700, make_pkt(0, 2)))
            fillers.append((1700, make_pkt(0, 1)))
            fillers.append((1700, make_pkt(0, 0)))

            debt = [0]
            opq = []
            pending = [None]
            n_b1 = len(fillers) - 6
            attention(0, fillers, debt, opq, pending,
                      due={2: n_b1 + 5, 6: n_b1 + 4, 10: n_b1 + 3,
                           13: n_b1 + 2, 25: n_b1 + 1, 33: n_b1})
            flush_fillers(fillers, pending, keep=21)
            attention(1, fillers, debt, opq, pending,
                      due={2: 15, 6: 9, 10: 3, 13: 2, 25: 1, 33: 0})
            flush_fillers(fillers, pending)
            while opq:
                outproj_item(*opq.pop(0))

    nc.compile()
    _NC_CACHE[key] = nc
    return nc


def _check_causal(mask: np.ndarray) -> bool:
    m = np.asarray(mask).reshape(mask.shape[-2], mask.shape[-1])
    s = m.shape[0]
    if np.array_equal(m, np.tril(np.ones((s, s), dtype=bool))):
        return True
    if m.all():
        return False
    raise NotImplementedError("only causal or all-true masks are supported")


def kernel(inputs_q, mask, Wq, bq, Wk, bk, Wv, bv, Wo, bo, _trace=False,
           _trace_cores=None):
    import ml_dtypes
    bf16 = ml_dtypes.bfloat16

    inputs_q = np.asarray(inputs_q, dtype=np.float32)
    Wq = np.asarray(Wq, dtype=np.float32).reshape(D, H * HD)
    Wk = np.asarray(Wk, dtype=np.float32).reshape(D, H * HD)
    Wv = np.asarray(Wv, dtype=np.float32).reshape(D, H * HD)
    Wo = np.asarray(Wo, dtype=np.float32).reshape(H * HD, D)
    bq = np.asarray(bq, dtype=np.float32).reshape(H * HD)
    bk = np.asarray(bk, dtype=np.float32).reshape(H * HD)
    bv = np.asarray(bv, dtype=np.float32).reshape(H * HD)
    bo = np.asarray(bo, dtype=np.float32).reshape(D)

    causal = _check_causal(mask)
    with_bias_qkv = bool(bq.any() or bk.any() or bv.any())

    nc = _build_nc(with_bias_qkv, causal)

    xtb = np.ascontiguousarray(inputs_q.transpose(0, 2, 1)).astype(bf16)
    in_maps = []
    for c in range(NCORES):
        f0, f1 = c * HH, (c + 1) * HH
        m = {
            "xt": xtb,
            "wq": np.ascontiguousarray(
                Wq[:, f0:f1].reshape(DC, P, HH).transpose(1, 0, 2)
            ).astype(bf16),
            "wk": np.ascontiguousarray(
                Wk[:, f0:f1].reshape(DC, P, HH).transpose(1, 0, 2)
            ).astype(bf16),
            "wv": np.ascontiguousarray(
                Wv[:, f0:f1].reshape(DC, P, HH).transpose(1, 0, 2)
            ).astype(bf16),
            "wo": np.ascontiguousarray(Wo[f0:f1, :]).astype(bf16),
        }
        if with_bias_qkv:
            m["bqkv"] = np.ascontiguousarray(
                np.stack([bq[f0:f1], bk[f0:f1], bv[f0:f1]])
            )
        in_maps.append(m)

    kwargs = {}
    if _trace:
        kwargs["trace"] = True
        if _trace_cores is not None:
            kwargs["trace_cores"] = _trace_cores
    res = run_bass_kernel_spmd(nc, in_maps, core_ids=list(range(NCORES)), **kwargs)

    acc = np.zeros((B, S, D), dtype=np.float64)
    for c in range(NCORES):
        acc += np.asarray(res.results[c]["out"], dtype=np.float64)
    acc += bo
    out = acc.astype(np.float32)
    if _trace:
        return out, res
    return out


# revision 20
# speedup vs baseline: 1.0237x; 1.0237x over previous
"""Causal multi-head attention (B=2, S=2048, D=1024, H=16, HD=64) on 8 trn2 cores.

Sharding: 2 heads per core x both batches (head-parallel QKV/attention/out-proj,
Wo h-split => per-core partial outputs, summed on host).

bf16 datapath (X/W/Q/K/V/e/otn bf16; projection + attnV accumulation and
softmax denominators fp32; scores and out-proj PSUM bf16 since they are
single-shot) — measured l2 ~6e-3 vs fp64 on CPU, well under the 2e-2 gate.
bf16 halves SBUF so both batches' X^T stream in from kernel start, and makes
N<256 matmuls run at 1 cycle/row so diagonal chunks are width-trimmed.

The PE HAM clock-gate re-throttles to 1.2GHz when it sees idle windows (the
previous kernel ran ~half its span at 1.2GHz), so everything is structured to
keep the PE stream dense:
  - scores run one group (= one k-chunk, both heads row-tiled at partition
    bases 0/64) ahead of the exp that consumes them (2-slot PSUM ring)
  - all projections are 8-matmul packets through a small PSUM ring; batch 0's
    run at kernel start (input DMA is column-block-major so packet 0 starts
    ~1.5us in), batch 1's sit in a cost-tracked filler queue drained into
    batch 0's attention slack (exp on ACT is the per-group limiter)
  - out-proj is deferred one item per group into the following q-tile's
    attention instead of bursting at the q-tile boundary
  - causal masking is a tiny DVE triangle-multiply on the 128-wide diagonal
    band (GPSIMD affine_select was 1.4us inside the score->exp->attnV chain)
  - softmax denominators ride the attnV matmul as a 65th ones-column in V^T;
    per q-tile one K=2 matmul broadcasts both heads' reciprocals

Engine budget per core: PE ~100us (proj 41 + scores 14 + attnV 29 + outproj
14 + transposes), ACT ~105us (exp 92 = the softmax floor + proj/out copies),
DVE ~65us (other PSUM evacuation, norm, masks), GPSIMD idle.

PSUM (8 banks): scores ring 2x[128,2,512]bf16 (2), o ring 4x[65,512]f32
(attnV accum + denominators) (4), small ring 2x 1-bank (proj packets f32,
transposes, bc, out-proj pairs bf16) (2).
"""

import numpy as np

import concourse.bass as bass
import concourse.mybir as mybir
import concourse.tile as tile
from concourse import bacc
from concourse.bass_utils import run_bass_kernel_spmd
from concourse.masks import make_identity, make_upper_triangular
from concourse.dve_ops import (RECIP_APPROX_FAST_CONSTS,
                               RECIPROCAL_APPROX_FAST)

F32 = mybir.dt.float32
F32R = mybir.dt.float32r
BF16 = mybir.dt.bfloat16
AF = mybir.ActivationFunctionType

B, S, D, H, HD = 2, 2048, 1024, 16, 64
NCORES = 8
HPC = H // NCORES          # heads per core = 2
HH = HPC * HD              # 128 concat head dims per core
P = 128
DC = D // P                # 8 d-chunks
NQ = 512                   # q tile (psum bank width fp32)
QJ = S // NQ               # 4 q tiles
KC = S // P                # 16 k chunks

_NC_CACHE = {}


def _build_nc(with_bias_qkv: bool, causal: bool):
    key = (with_bias_qkv, causal)
    if key in _NC_CACHE:
        return _NC_CACHE[key]

    nc = bacc.Bacc("TRN2", target_bir_lowering=False, debug=False)
    xt = nc.dram_tensor("xt", [B, D, S], BF16, kind="ExternalInput")
    wq = nc.dram_tensor("wq", [P, DC, HH], BF16, kind="ExternalInput")
    wk = nc.dram_tensor("wk", [P, DC, HH], BF16, kind="ExternalInput")
    wv = nc.dram_tensor("wv", [P, DC, HH], BF16, kind="ExternalInput")
    wo = nc.dram_tensor("wo", [HH, D], BF16, kind="ExternalInput")
    if with_bias_qkv:
        bqkv = nc.dram_tensor("bqkv", [3, HH], F32, kind="ExternalInput")
    out = nc.dram_tensor("out", [B, S, D], BF16, kind="ExternalOutput")

    with tile.TileContext(nc) as tc:
        with (
            tc.tile_pool(name="const", bufs=1) as cpool,
            tc.tile_pool(name="xtp", bufs=1) as xt_pool,
            tc.tile_pool(name="qkv", bufs=1) as qkv_pool,
            tc.tile_pool(name="ep", bufs=3) as e_pool,
            tc.tile_pool(name="nrm", bufs=2) as nrm_pool,
            tc.tile_pool(name="osb", bufs=4) as osb_pool,
            tc.tile_pool(name="psb", bufs=2, space="PSUM") as ps_big,
            tc.tile_pool(name="pss", bufs=2, space="PSUM") as ps_sm,
            tc.tile_pool(name="pso", bufs=2, space="PSUM") as ps_o,
        ):
            # ---- weights + X^T DMA, ordered so V packet j0 can start early
            wv_sb = cpool.tile([P, DC, HH], BF16, tag="wv", name="wv_sb")
            nc.sync.dma_start(wv_sb[:], wv[:])
            if with_bias_qkv:
                bqkvt_sb = cpool.tile([HH, 3], F32, tag="bqkvt", name="bqkvt_sb")
                for i in range(3):
                    nc.sync.dma_start(
                        bqkvt_sb[:, i:i + 1], bqkv[i:i + 1, :].rearrange("a f -> f a")
                    )
            xt_sb = [None, None]
            for b in range(B):
                xt_sb[b] = xt_pool.tile([P, DC, S], BF16, tag=f"xt{b}",
                                        name=f"xt{b}_sb")

            def load_xt(b, j):
                nc.sync.dma_start(
                    xt_sb[b][:, :, j * NQ:(j + 1) * NQ],
                    xt[b, :, j * NQ:(j + 1) * NQ].rearrange(
                        "(o p) n -> p o n", p=P),
                )

            load_xt(0, 0)
            load_xt(0, 1)
            wq_sb = cpool.tile([P, DC, HH], BF16, tag="wq", name="wq_sb")
            nc.sync.dma_start(wq_sb[:], wq[:])
            wk_sb = cpool.tile([P, DC, HH], BF16, tag="wk", name="wk_sb")
            nc.sync.dma_start(wk_sb[:], wk[:])
            load_xt(0, 2)
            load_xt(0, 3)
            wo_sb = cpool.tile([P, D], BF16, tag="wo", name="wo_sb")
            nc.sync.dma_start(wo_sb[:], wo[:])
            for j in range(QJ):
                load_xt(1, j)

            ident_sb = cpool.tile([P, P], BF16, tag="ident", name="ident_sb")
            make_identity(nc, ident_sb[:])
            # mtri[p, j] = 1 if j >= p else 0: causal keep-mask for the
            # 128-wide diagonal band of a k-chunk
            mtri_sb = cpool.tile([P, P], BF16, tag="mtri", name="mtri_sb")
            make_upper_triangular(nc, mtri_sb[:], val=1.0, diag=True)
            # ones2: row 0 selects head-0 partitions, row 32 head-1 (engine
            # partition bases must be multiples of 32) -- one K=33 matmul
            # broadcasts both heads' 1/denom over their 64 partitions
            ones2_sb = cpool.tile([33, P], F32R, tag="ones2", name="ones2_sb")
            nc.vector.memset(ones2_sb[:].bitcast(F32), 0.0)
            nc.vector.memset(ones2_sb[0:1, 0:HD].bitcast(F32), 1.0)
            nc.vector.memset(ones2_sb[32:33, HD:HH].bitcast(F32), 1.0)

            # norm scratch: persistent; dead rows 1-31 kept finite once
            rsq_sb = cpool.tile([33, NQ], F32, tag="rsq", name="rsq_sb")
            nc.vector.memset(rsq_sb[:], 1.0)
            rinq_sb = cpool.tile([33, NQ], F32R, tag="rinq", name="rinq_sb")

            warm_sb = cpool.tile([P, NQ], BF16, tag="warm", name="warm_sb")
            nc.vector.memset(warm_sb[:], 0.0)

            def pe_warmup(n):
                wp = ps_sm.tile([P, NQ], F32, tag="sm", name="warm_ps")
                for i in range(n):
                    nc.tensor.matmul(wp[:], lhsT=ident_sb[:], rhs=warm_sb[:],
                                     start=(i == 0), stop=(i == n - 1))

            # per-batch state
            st = [dict() for _ in range(B)]

            def alloc_qkv(b):
                st[b]["qt"] = qkv_pool.tile([P, QJ, NQ], BF16, tag=f"qt{b}",
                                            name="qt_sb")
                st[b]["kt"] = qkv_pool.tile([P, QJ, NQ], BF16, tag=f"kt{b}",
                                            name="kt_sb")
                st[b]["vt"] = qkv_pool.tile([P, QJ, NQ], BF16, tag="vt",
                                            name="vt_sb")
                v_sb = qkv_pool.tile([P, KC, HPC, HD + 2], BF16, tag=f"v{b}",
                                     name="v_sb")
                # 65th lhsT column = 1.0 -> attnV also accumulates sum(e)
                nc.vector.memset(v_sb[:, :, :, HD:HD + 1], 1.0)
                st[b]["v"] = v_sb

            def proj_packet(b, w_idx, j, half=None):
                """One NQ block of one projection: 8 accumulating matmuls
                (fp32 psum) through the small ring + one PSUM->SBUF copy
                (ACT for batch 0 / DVE for batch 1, balancing the phases).
                half=0 runs d 0-3 and returns the psum tile; half=(1, pps)
                finishes d 4-7 + copy."""
                w_sb = (wq_sb, wk_sb, wv_sb)[w_idx]
                dst = (st[b]["qt"], st[b]["kt"], st[b]["vt"])[w_idx]
                if half is None:
                    pps = ps_sm.tile([P, NQ], F32, tag="sm", name="pkt_ps")
                    dr = range(DC)
                elif half == 0:
                    pps = ps_sm.tile([P, NQ], F32, tag="sm", name="pkt_ps")
                    dr = range(DC // 2)
                else:
                    pps = half[1]
                    dr = range(DC // 2, DC)
                for d in dr:
                    nc.tensor.matmul(
                        pps[:],
                        lhsT=w_sb[:, d, :],
                        rhs=xt_sb[b][:, d, j * NQ:(j + 1) * NQ],
                        start=(d == 0), stop=(d == DC - 1),
                    )
                if half == 0:
                    return pps
                if with_bias_qkv:
                    if b == 0:
                        nc.scalar.activation(
                            dst[:, j, :], pps[:], AF.Identity,
                            bias=bqkvt_sb[:, w_idx:w_idx + 1],
                        )
                    else:
                        nc.vector.tensor_scalar_add(
                            dst[:, j, :], pps[:], bqkvt_sb[:, w_idx:w_idx + 1]
                        )
                elif b == 0:
                    nc.scalar.copy(dst[:, j, :], pps[:])
                else:
                    nc.vector.tensor_copy(dst[:, j, :], pps[:])

            def v_transpose(b, sc):
                tp = ps_sm.tile([P, P], BF16, tag="sm", name="tr_ps")
                nc.tensor.transpose(
                    tp[:], st[b]["vt"][:, sc // 4, (sc % 4) * P:(sc % 4 + 1) * P],
                    ident_sb[:],
                )
                # both heads in one strided copy: [128,128] -> [128, 2, 64]
                nc.vector.tensor_copy(
                    st[b]["v"][:, sc, :, 0:HD],
                    tp[:].rearrange("p (h d) -> p h d", h=HPC),
                )

            # ------------- attention -------------
            def group_seq():
                for qj in (3, 2, 1, 0):
                    nk = 4 * (qj + 1) if causal else KC
                    for ki in range(nk):
                        yield qj, ki, nk

            def emit_scores(b, qj, ki):
                qt, kt = st[b]["qt"], st[b]["kt"]
                sp = ps_big.tile([P, 2, NQ], F32, tag="big", name="st_ps")
                o = ki - 4 * qj if causal else -1
                q0 = o * P if o > 0 else 0
                for h in range(HPC):
                    h0 = h * HD
                    nc.tensor.matmul(
                        sp[:, h, q0:NQ],
                        lhsT=kt[h0:h0 + HD, ki // 4, (ki % 4) * P:(ki % 4 + 1) * P],
                        rhs=qt[h0:h0 + HD, qj, q0:NQ],
                        start=True, stop=True,
                    )
                return sp

            def emit_exp(b, qj, ki, sp):
                e_sb = e_pool.tile([P, 2, NQ], BF16, tag="e", name="e_sb")
                o = ki - 4 * qj if causal else -1
                q0 = o * P if o > 0 else 0
                nc.scalar.activation(e_sb[:, :, q0:NQ], sp[:, :, q0:NQ],
                                     AF.Exp, scale=0.125)
                if causal and o >= 0:
                    # zero the masked triangle of the 128-wide diagonal band
                    for h in range(HPC):
                        nc.vector.tensor_mul(
                            e_sb[:, h, o * P:(o + 1) * P],
                            e_sb[:, h, o * P:(o + 1) * P],
                            mtri_sb[:],
                        )
                return e_sb

            def emit_attnv(b, qj, ki, nk, e_sb, o_ps):
                v = st[b]["v"]
                o = ki - 4 * qj if causal else -1
                q0 = o * P if o > 0 else 0
                for h in range(HPC):
                    nc.tensor.matmul(
                        o_ps[h][:, q0:NQ],
                        lhsT=v[:, ki, h, 0:HD + 1],
                        rhs=e_sb[:, h, q0:NQ],
                        start=(ki == 0), stop=(ki == nk - 1),
                    )

            def emit_norm(b, qj, o_ps):
                otn = st[b]["otn"]
                for h in range(HPC):
                    nc.vector.tensor_copy(rsq_sb[32 * h:32 * h + 1, :],
                                          o_ps[h][HD:HD + 1, :])
                c = RECIP_APPROX_FAST_CONSTS
                nc.vector._custom_dve(
                    RECIPROCAL_APPROX_FAST, out=rinq_sb[:], in0=rsq_sb[:],
                    s0=c["s0"], s1=c["s1"], imm2=c["imm2"],
                )
                bc_ps = ps_sm.tile([P, NQ], F32, tag="sm", name="bc_ps")
                nc.tensor.matmul(bc_ps[:], lhsT=ones2_sb[:], rhs=rinq_sb[:],
                                 start=True, stop=True)
                # DVE reads at most one PSUM operand: stage o through SBUF
                ot = nrm_pool.tile([HH, NQ], F32, tag="ot", name="ot")
                for h in range(HPC):
                    h0 = h * HD
                    nc.vector.tensor_copy(ot[h0:h0 + HD, :], o_ps[h][0:HD, :])
                for h in range(HPC):
                    h0 = h * HD
                    nc.vector.tensor_mul(
                        otn[h0:h0 + HD, qj, :], ot[h0:h0 + HD, :],
                        bc_ps[h0:h0 + HD, :]
                    )

            def outproj_item(b, qj, sc4, _ct=[0]):
                otn = st[b]["otn"]
                sc = qj * 4 + sc4
                out_sb = osb_pool.tile([P, 2, NQ], BF16, tag="out", name="out_sb")
                for fc in range(2):
                    ops = ps_sm.tile([P, NQ], F32, tag="sm", name="op_ps")
                    nc.tensor.matmul(
                        ops[:],
                        lhsT=otn[:, qj, sc4 * P:(sc4 + 1) * P],
                        rhs=wo_sb[:, fc * NQ:(fc + 1) * NQ],
                        start=True, stop=True,
                    )
                    if _ct[0] % 5 < 2:
                        nc.scalar.copy(out_sb[:, fc, :], ops[:])
                    else:
                        nc.vector.tensor_copy(out_sb[:, fc, :], ops[:])
                    _ct[0] += 1
                nc.sync.dma_start(
                    out[b, sc * P:(sc + 1) * P, :],
                    out_sb.rearrange("p a n -> p (a n)"),
                )

            def flush_fillers(fillers, pending, keep=0):
                while len(fillers) > keep or pending[0] is not None:
                    if pending[0] is not None:
                        fn, pending[0] = pending[0], None
                        fn()
                    else:
                        pending[0] = fillers.pop(0)[1]()

            def attention(b, fillers, fill_debt, opq, pending, due=()):
                st[b]["otn"] = qkv_pool.tile([P, QJ, NQ], BF16, tag=f"otn{b}",
                                             name="otn_sb")
                due = dict(due)
                seq = list(group_seq())
                sp_next = emit_scores(b, *seq[0][:2])
                o_ps = None
                for i, (qj, ki, nk) in enumerate(seq):
                    if i in due:
                        flush_fillers(fillers, pending, keep=due[i])
                    if ki == 0:
                        o_ps = [ps_o.tile([HD + 1, NQ], F32, tag="o",
                                          name=f"o_ps{h}") for h in range(HPC)]
                    sp = sp_next
                    if i + 1 < len(seq):
                        sp_next = emit_scores(b, *seq[i + 1][:2])
                    e_sb = emit_exp(b, qj, ki, sp)
                    emit_attnv(b, qj, ki, nk, e_sb, o_ps)
                    if ki == nk - 1:
                        emit_norm(b, qj, o_ps)
                        for sc4 in range(4):
                            opq.append((b, qj, sc4))
                    elif pending[0] is not None:
                        fn, pending[0] = pending[0], None
                        fn()
                    elif opq:
                        outproj_item(*opq.pop(0))
                    # exp on ACT (~0.93us avg) outpaces the group's PE work
                    # (~0.65us): drain the filler queue with the difference
                    fill_debt[0] += 420
                    while (pending[0] is None and fillers
                           and fill_debt[0] >= fillers[0][0]):
                        cost, fn = fillers.pop(0)
                        fill_debt[0] -= cost
                        pending[0] = fn()

            # ---------------- program ----------------
            pe_warmup(18)
            alloc_qkv(0)
            # batch-0 projections: V first (j 0..3) so transposes interleave,
            # Q in j 3..0 (attention consumes q-tile 3 first), K in j 0..3
            p0 = [(2, 0), (2, 1)]
            p0 += [("t", 0), ("t", 1), (2, 2), ("t", 2), ("t", 3), (2, 3)]
            p0 += [(0, 3), ("t", 4), ("t", 5), (0, 2), ("t", 6), ("t", 7)]
            p0 += [(0, 1), ("t", 8), ("t", 9), (0, 0), ("t", 10), ("t", 11)]
            p0 += [(1, 0), ("t", 12), ("t", 13), (1, 1), ("t", 14), ("t", 15)]
            p0 += [(1, 2), (1, 3)]
            for w_idx, j in p0:
                if w_idx == "t":
                    v_transpose(0, j)
                else:
                    proj_packet(0, w_idx, j)

            alloc_qkv(1)
            # filler order: V packets feed transposes; Q packets in j 3..0
            # (b1 attention reads q-tile 3 first), K packets in j 0..3.
            # Packets split in two 4-matmul halves; half A returns half B,
            # which MUST be the next queue-driven small-ring consumer.
            def make_pkt(w, j, b=1):
                def half_a():
                    pps = proj_packet(b, w, j, half=0)
                    return lambda: proj_packet(b, w, j, half=(1, pps))
                return half_a

            fillers = []
            def tr1(s):
                return (250, lambda ss=s: v_transpose(1, ss) and None)

            # batch-1 order: [Q3, V0, tr0-3, K0] unblock its first groups
            # (flushed at the bridge); then V/tr/K waves sized to the groups
            # that consume them, Q j2/j1/j0 last -- all due-date forced
            fillers.append((1700, make_pkt(0, 3)))
            fillers.append((1700, make_pkt(2, 0)))
            fillers += [tr1(0), tr1(1), tr1(2), tr1(3)]
            fillers.append((1700, make_pkt(1, 0)))
            fillers.append((1700, make_pkt(2, 1)))
            fillers += [tr1(4), tr1(5), tr1(6), tr1(7)]
            fillers.append((1700, make_pkt(1, 1)))
            fillers.append((1700, make_pkt(2, 2)))
            fillers += [tr1(8), tr1(9), tr1(10), tr1(11)]
            fillers.append((1700, make_pkt(1, 2)))
            fillers.append((1700, make_pkt(2, 3)))
            fillers += [tr1(12), tr1(13), tr1(14), tr1(15)]
            fillers.append((1700, make_pkt(1, 3)))
            fillers.append((1700, make_pkt(0, 2)))
            fillers.append((1700, make_pkt(0, 1)))
            fillers.append((1700, make_pkt(0, 0)))

            debt = [0]
            opq = []
            pending = [None]
            attention(0, fillers, debt, opq, pending)
            flush_fillers(fillers, pending, keep=21)
            attention(1, fillers, debt, opq, pending,
                      due={2: 15, 6: 9, 10: 3, 13: 2, 25: 1, 33: 0})
            flush_fillers(fillers, pending)
            while opq:
                outproj_item(*opq.pop(0))

    nc.compile()
    _NC_CACHE[key] = nc
    return nc


def _check_causal(mask: np.ndarray) -> bool:
    m = np.asarray(mask).reshape(mask.shape[-2], mask.shape[-1])
    s = m.shape[0]
    if np.array_equal(m, np.tril(np.ones((s, s), dtype=bool))):
        return True
    if m.all():
        return False
    raise NotImplementedError("only causal or all-true masks are supported")


def kernel(inputs_q, mask, Wq, bq, Wk, bk, Wv, bv, Wo, bo, _trace=False,
           _trace_cores=None):
    import ml_dtypes
    bf16 = ml_dtypes.bfloat16

    inputs_q = np.asarray(inputs_q, dtype=np.float32)
    Wq = np.asarray(Wq, dtype=np.float32).reshape(D, H * HD)
    Wk = np.asarray(Wk, dtype=np.float32).reshape(D, H * HD)
    Wv = np.asarray(Wv, dtype=np.float32).reshape(D, H * HD)
    Wo = np.asarray(Wo, dtype=np.float32).reshape(H * HD, D)
    bq = np.asarray(bq, dtype=np.float32).reshape(H * HD)
    bk = np.asarray(bk, dtype=np.float32).reshape(H * HD)
    bv = np.asarray(bv, dtype=np.float32).reshape(H * HD)
    bo = np.asarray(bo, dtype=np.float32).reshape(D)

    causal = _check_causal(mask)
    with_bias_qkv = bool(bq.any() or bk.any() or bv.any())

    nc = _build_nc(with_bias_qkv, causal)

    xtb = np.ascontiguousarray(inputs_q.transpose(0, 2, 1)).astype(bf16)
    in_maps = []
    for c in range(NCORES):
        f0, f1 = c * HH, (c + 1) * HH
        m = {
            "xt": xtb,
            "wq": np.ascontiguousarray(
                Wq[:, f0:f1].reshape(DC, P, HH).transpose(1, 0, 2)
            ).astype(bf16),
            "wk": np.ascontiguousarray(
                Wk[:, f0:f1].reshape(DC, P, HH).transpose(1, 0, 2)
            ).astype(bf16),
            "wv": np.ascontiguousarray(
                Wv[:, f0:f1].reshape(DC, P, HH).transpose(1, 0, 2)
            ).astype(bf16),
            "wo": np.ascontiguousarray(Wo[f0:f1, :]).astype(bf16),
        }
        if with_bias_qkv:
            m["bqkv"] = np.ascontiguousarray(
                np.stack([bq[f0:f1], bk[f0:f1], bv[f0:f1]])
            )
        in_maps.append(m)

    kwargs = {}
    if _trace:
        kwargs["trace"] = True
        if _trace_cores is not None:
            kwargs["trace_cores"] = _trace_cores
    res = run_bass_kernel_spmd(nc, in_maps, core_ids=list(range(NCORES)), **kwargs)

    acc = np.zeros((B, S, D), dtype=np.float64)
    for c in range(NCORES):
        acc += np.asarray(res.results[c]["out"], dtype=np.float64)
    acc += bo
    out = acc.astype(np.float32)
    if _trace:
        return out, res
    return out


# revision 21
# speedup vs baseline: 1.0391x; 1.0150x over previous
"""Causal multi-head attention (B=2, S=2048, D=1024, H=16, HD=64) on 8 trn2 cores.

Sharding: 2 heads per core x both batches (head-parallel QKV/attention/out-proj,
Wo h-split => per-core partial outputs, summed on host).

bf16 datapath (X/W/Q/K/V/e/otn bf16; projection + attnV accumulation and
softmax denominators fp32; scores and out-proj PSUM bf16 since they are
single-shot) — measured l2 ~6e-3 vs fp64 on CPU, well under the 2e-2 gate.
bf16 halves SBUF so both batches' X^T stream in from kernel start, and makes
N<256 matmuls run at 1 cycle/row so diagonal chunks are width-trimmed.

The PE HAM clock-gate re-throttles to 1.2GHz when it sees idle windows (the
previous kernel ran ~half its span at 1.2GHz), so everything is structured to
keep the PE stream dense:
  - scores run one group (= one k-chunk, both heads row-tiled at partition
    bases 0/64) ahead of the exp that consumes them (2-slot PSUM ring)
  - all projections are 8-matmul packets through a small PSUM ring; batch 0's
    run at kernel start (input DMA is column-block-major so packet 0 starts
    ~1.5us in), batch 1's sit in a cost-tracked filler queue drained into
    batch 0's attention slack (exp on ACT is the per-group limiter)
  - out-proj is deferred one item per group into the following q-tile's
    attention instead of bursting at the q-tile boundary
  - causal masking is a tiny DVE triangle-multiply on the 128-wide diagonal
    band (GPSIMD affine_select was 1.4us inside the score->exp->attnV chain)
  - softmax denominators ride the attnV matmul as a 65th ones-column in V^T;
    per q-tile one K=2 matmul broadcasts both heads' reciprocals

Engine budget per core: PE ~100us (proj 41 + scores 14 + attnV 29 + outproj
14 + transposes), ACT ~105us (exp 92 = the softmax floor + proj/out copies),
DVE ~65us (other PSUM evacuation, norm, masks), GPSIMD idle.

PSUM (8 banks): scores ring 2x[128,2,512]bf16 (2), o ring 4x[65,512]f32
(attnV accum + denominators) (4), small ring 2x 1-bank (proj packets f32,
transposes, bc, out-proj pairs bf16) (2).
"""

import numpy as np

import concourse.bass as bass
import concourse.mybir as mybir
import concourse.tile as tile
from concourse import bacc
from concourse.bass_utils import run_bass_kernel_spmd
from concourse.masks import make_identity, make_upper_triangular
from concourse.dve_ops import (RECIP_APPROX_FAST_CONSTS,
                               RECIPROCAL_APPROX_FAST)

F32 = mybir.dt.float32
F32R = mybir.dt.float32r
BF16 = mybir.dt.bfloat16
AF = mybir.ActivationFunctionType

B, S, D, H, HD = 2, 2048, 1024, 16, 64
NCORES = 8
HPC = H // NCORES          # heads per core = 2
HH = HPC * HD              # 128 concat head dims per core
P = 128
DC = D // P                # 8 d-chunks
NQ = 512                   # q tile (psum bank width fp32)
QJ = S // NQ               # 4 q tiles
KC = S // P                # 16 k chunks

_NC_CACHE = {}


def _build_nc(with_bias_qkv: bool, causal: bool):
    key = (with_bias_qkv, causal)
    if key in _NC_CACHE:
        return _NC_CACHE[key]

    nc = bacc.Bacc("TRN2", target_bir_lowering=False, debug=False)
    xt = nc.dram_tensor("xt", [B, D, S], BF16, kind="ExternalInput")
    wq = nc.dram_tensor("wq", [P, DC, HH], BF16, kind="ExternalInput")
    wk = nc.dram_tensor("wk", [P, DC, HH], BF16, kind="ExternalInput")
    wv = nc.dram_tensor("wv", [P, DC, HH], BF16, kind="ExternalInput")
    wo = nc.dram_tensor("wo", [HH, D], BF16, kind="ExternalInput")
    if with_bias_qkv:
        bqkv = nc.dram_tensor("bqkv", [3, HH], F32, kind="ExternalInput")
    out = nc.dram_tensor("out", [B, S, D], BF16, kind="ExternalOutput")

    with tile.TileContext(nc) as tc:
        with (
            tc.tile_pool(name="const", bufs=1) as cpool,
            tc.tile_pool(name="xtp", bufs=1) as xt_pool,
            tc.tile_pool(name="qkv", bufs=1) as qkv_pool,
            tc.tile_pool(name="ep", bufs=3) as e_pool,
            tc.tile_pool(name="nrm", bufs=2) as nrm_pool,
            tc.tile_pool(name="osb", bufs=4) as osb_pool,
            tc.tile_pool(name="psb", bufs=2, space="PSUM") as ps_big,
            tc.tile_pool(name="pss", bufs=2, space="PSUM") as ps_sm,
            tc.tile_pool(name="pso", bufs=2, space="PSUM") as ps_o,
        ):
            # ---- weights + X^T DMA, ordered so V packet j0 can start early
            wv_sb = cpool.tile([P, DC, HH], BF16, tag="wv", name="wv_sb")
            nc.sync.dma_start(wv_sb[:], wv[:])
            if with_bias_qkv:
                bqkvt_sb = cpool.tile([HH, 3], F32, tag="bqkvt", name="bqkvt_sb")
                for i in range(3):
                    nc.sync.dma_start(
                        bqkvt_sb[:, i:i + 1], bqkv[i:i + 1, :].rearrange("a f -> f a")
                    )
            xt_sb = [None, None]
            for b in range(B):
                xt_sb[b] = xt_pool.tile([P, DC, S], BF16, tag=f"xt{b}",
                                        name=f"xt{b}_sb")

            def load_xt(b, j):
                nc.sync.dma_start(
                    xt_sb[b][:, :, j * NQ:(j + 1) * NQ],
                    xt[b, :, j * NQ:(j + 1) * NQ].rearrange(
                        "(o p) n -> p o n", p=P),
                )

            load_xt(0, 0)
            load_xt(0, 1)
            wq_sb = cpool.tile([P, DC, HH], BF16, tag="wq", name="wq_sb")
            nc.sync.dma_start(wq_sb[:], wq[:])
            wk_sb = cpool.tile([P, DC, HH], BF16, tag="wk", name="wk_sb")
            nc.sync.dma_start(wk_sb[:], wk[:])
            load_xt(0, 2)
            load_xt(0, 3)
            wo_sb = cpool.tile([P, D], BF16, tag="wo", name="wo_sb")
            nc.sync.dma_start(wo_sb[:], wo[:])
            for j in range(QJ):
                load_xt(1, j)

            ident_sb = cpool.tile([P, P], BF16, tag="ident", name="ident_sb")
            make_identity(nc, ident_sb[:])
            # mtri[p, j] = 1 if j >= p else 0: causal keep-mask for the
            # 128-wide diagonal band of a k-chunk
            mtri_sb = cpool.tile([P, P], BF16, tag="mtri", name="mtri_sb")
            make_upper_triangular(nc, mtri_sb[:], val=1.0, diag=True)
            # ones2: row 0 selects head-0 partitions, row 32 head-1 (engine
            # partition bases must be multiples of 32) -- one K=33 matmul
            # broadcasts both heads' 1/denom over their 64 partitions
            ones2_sb = cpool.tile([33, P], F32R, tag="ones2", name="ones2_sb")
            nc.vector.memset(ones2_sb[:].bitcast(F32), 0.0)
            nc.vector.memset(ones2_sb[0:1, 0:HD].bitcast(F32), 1.0)
            nc.vector.memset(ones2_sb[32:33, HD:HH].bitcast(F32), 1.0)

            # norm scratch: persistent; dead rows 1-31 kept finite once
            rsq_sb = cpool.tile([33, NQ], F32, tag="rsq", name="rsq_sb")
            nc.vector.memset(rsq_sb[:], 1.0)
            rinq_sb = cpool.tile([33, NQ], F32R, tag="rinq", name="rinq_sb")

            warm_sb = cpool.tile([P, NQ], BF16, tag="warm", name="warm_sb")
            nc.vector.memset(warm_sb[:], 0.0)

            def pe_warmup(n):
                wp = ps_sm.tile([P, NQ], F32, tag="sm", name="warm_ps")
                for i in range(n):
                    nc.tensor.matmul(wp[:], lhsT=ident_sb[:], rhs=warm_sb[:],
                                     start=(i == 0), stop=(i == n - 1))

            def pe_warmup_big(n):
                wp = ps_big.tile([P, 2, NQ], F32, tag="big", name="warmb_ps")
                for i in range(n):
                    nc.tensor.matmul(wp[:, 0, :], lhsT=ident_sb[:],
                                     rhs=warm_sb[:],
                                     start=(i == 0), stop=(i == n - 1))

            # per-batch state
            st = [dict() for _ in range(B)]

            def alloc_qkv(b):
                st[b]["qt"] = qkv_pool.tile([P, QJ, NQ], BF16, tag=f"qt{b}",
                                            name="qt_sb")
                st[b]["kt"] = qkv_pool.tile([P, QJ, NQ], BF16, tag=f"kt{b}",
                                            name="kt_sb")
                st[b]["vt"] = qkv_pool.tile([P, QJ, NQ], BF16, tag="vt",
                                            name="vt_sb")
                v_sb = qkv_pool.tile([P, KC, HPC, HD + 2], BF16, tag=f"v{b}",
                                     name="v_sb")
                # 65th lhsT column = 1.0 -> attnV also accumulates sum(e)
                nc.vector.memset(v_sb[:, :, :, HD:HD + 1], 1.0)
                st[b]["v"] = v_sb

            def proj_packet(b, w_idx, j, half=None):
                """One NQ block of one projection: 8 accumulating matmuls
                (fp32 psum) through the small ring + one PSUM->SBUF copy
                (ACT for batch 0 / DVE for batch 1, balancing the phases).
                half=0 runs d 0-3 and returns the psum tile; half=(1, pps)
                finishes d 4-7 + copy."""
                w_sb = (wq_sb, wk_sb, wv_sb)[w_idx]
                dst = (st[b]["qt"], st[b]["kt"], st[b]["vt"])[w_idx]
                if half is None:
                    pps = ps_sm.tile([P, NQ], F32, tag="sm", name="pkt_ps")
                    dr = range(DC)
                elif half == 0:
                    pps = ps_sm.tile([P, NQ], F32, tag="sm", name="pkt_ps")
                    dr = range(DC // 2)
                else:
                    pps = half[1]
                    dr = range(DC // 2, DC)
                for d in dr:
                    nc.tensor.matmul(
                        pps[:],
                        lhsT=w_sb[:, d, :],
                        rhs=xt_sb[b][:, d, j * NQ:(j + 1) * NQ],
                        start=(d == 0), stop=(d == DC - 1),
                    )
                if half == 0:
                    return pps
                if with_bias_qkv:
                    if b == 0:
                        nc.scalar.activation(
                            dst[:, j, :], pps[:], AF.Identity,
                            bias=bqkvt_sb[:, w_idx:w_idx + 1],
                        )
                    else:
                        nc.vector.tensor_scalar_add(
                            dst[:, j, :], pps[:], bqkvt_sb[:, w_idx:w_idx + 1]
                        )
                elif b == 0:
                    nc.scalar.copy(dst[:, j, :], pps[:])
                else:
                    nc.vector.tensor_copy(dst[:, j, :], pps[:])

            def v_transpose(b, sc):
                tp = ps_sm.tile([P, P], BF16, tag="sm", name="tr_ps")
                nc.tensor.transpose(
                    tp[:], st[b]["vt"][:, sc // 4, (sc % 4) * P:(sc % 4 + 1) * P],
                    ident_sb[:],
                )
                # both heads in one strided copy: [128,128] -> [128, 2, 64]
                nc.vector.tensor_copy(
                    st[b]["v"][:, sc, :, 0:HD],
                    tp[:].rearrange("p (h d) -> p h d", h=HPC),
                )

            # ------------- attention -------------
            def group_seq():
                for qj in (3, 2, 1, 0):
                    nk = 4 * (qj + 1) if causal else KC
                    for ki in range(nk):
                        yield qj, ki, nk

            def emit_scores(b, qj, ki):
                qt, kt = st[b]["qt"], st[b]["kt"]
                sp = ps_big.tile([P, 2, NQ], F32, tag="big", name="st_ps")
                o = ki - 4 * qj if causal else -1
                q0 = o * P if o > 0 else 0
                for h in range(HPC):
                    h0 = h * HD
                    nc.tensor.matmul(
                        sp[:, h, q0:NQ],
                        lhsT=kt[h0:h0 + HD, ki // 4, (ki % 4) * P:(ki % 4 + 1) * P],
                        rhs=qt[h0:h0 + HD, qj, q0:NQ],
                        start=True, stop=True,
                    )
                return sp

            def emit_exp(b, qj, ki, sp):
                e_sb = e_pool.tile([P, 2, NQ], BF16, tag="e", name="e_sb")
                o = ki - 4 * qj if causal else -1
                q0 = o * P if o > 0 else 0
                nc.scalar.activation(e_sb[:, :, q0:NQ], sp[:, :, q0:NQ],
                                     AF.Exp, scale=0.125)
                if causal and o >= 0:
                    # zero the masked triangle of the 128-wide diagonal band
                    for h in range(HPC):
                        nc.vector.tensor_mul(
                            e_sb[:, h, o * P:(o + 1) * P],
                            e_sb[:, h, o * P:(o + 1) * P],
                            mtri_sb[:],
                        )
                return e_sb

            def emit_attnv(b, qj, ki, nk, e_sb, o_ps):
                v = st[b]["v"]
                o = ki - 4 * qj if causal else -1
                q0 = o * P if o > 0 else 0
                for h in range(HPC):
                    nc.tensor.matmul(
                        o_ps[h][:, q0:NQ],
                        lhsT=v[:, ki, h, 0:HD + 1],
                        rhs=e_sb[:, h, q0:NQ],
                        start=(ki == 0), stop=(ki == nk - 1),
                    )

            def emit_norm(b, qj, o_ps):
                otn = st[b]["otn"]
                for h in range(HPC):
                    nc.vector.tensor_copy(rsq_sb[32 * h:32 * h + 1, :],
                                          o_ps[h][HD:HD + 1, :])
                c = RECIP_APPROX_FAST_CONSTS
                nc.vector._custom_dve(
                    RECIPROCAL_APPROX_FAST, out=rinq_sb[:], in0=rsq_sb[:],
                    s0=c["s0"], s1=c["s1"], imm2=c["imm2"],
                )
                bc_ps = ps_sm.tile([P, NQ], F32, tag="sm", name="bc_ps")
                nc.tensor.matmul(bc_ps[:], lhsT=ones2_sb[:], rhs=rinq_sb[:],
                                 start=True, stop=True)
                # DVE reads at most one PSUM operand: stage o through SBUF
                ot = nrm_pool.tile([HH, NQ], F32, tag="ot", name="ot")
                for h in range(HPC):
                    h0 = h * HD
                    nc.vector.tensor_copy(ot[h0:h0 + HD, :], o_ps[h][0:HD, :])
                for h in range(HPC):
                    h0 = h * HD
                    nc.vector.tensor_mul(
                        otn[h0:h0 + HD, qj, :], ot[h0:h0 + HD, :],
                        bc_ps[h0:h0 + HD, :]
                    )

            def outproj_item(b, qj, sc4, _ct=[0]):
                otn = st[b]["otn"]
                sc = qj * 4 + sc4
                out_sb = osb_pool.tile([P, 2, NQ], BF16, tag="out", name="out_sb")
                for fc in range(2):
                    ops = ps_sm.tile([P, NQ], F32, tag="sm", name="op_ps")
                    nc.tensor.matmul(
                        ops[:],
                        lhsT=otn[:, qj, sc4 * P:(sc4 + 1) * P],
                        rhs=wo_sb[:, fc * NQ:(fc + 1) * NQ],
                        start=True, stop=True,
                    )
                    if _ct[0] % 5 < 2:
                        nc.scalar.copy(out_sb[:, fc, :], ops[:])
                    else:
                        nc.vector.tensor_copy(out_sb[:, fc, :], ops[:])
                    _ct[0] += 1
                nc.sync.dma_start(
                    out[b, sc * P:(sc + 1) * P, :],
                    out_sb.rearrange("p a n -> p (a n)"),
                )

            def flush_fillers(fillers, pending, keep=0):
                while len(fillers) > keep or pending[0] is not None:
                    if pending[0] is not None:
                        fn, pending[0] = pending[0], None
                        fn()
                    else:
                        pending[0] = fillers.pop(0)[1]()

            def attention(b, fillers, fill_debt, opq, pending, due=()):
                st[b]["otn"] = qkv_pool.tile([P, QJ, NQ], BF16, tag=f"otn{b}",
                                             name="otn_sb")
                due = dict(due)
                seq = list(group_seq())
                sp_next = emit_scores(b, *seq[0][:2])
                o_ps = None
                for i, (qj, ki, nk) in enumerate(seq):
                    if i in due:
                        flush_fillers(fillers, pending, keep=due[i])
                    if ki == 0:
                        o_ps = [ps_o.tile([HD + 1, NQ], F32, tag="o",
                                          name=f"o_ps{h}") for h in range(HPC)]
                    sp = sp_next
                    if i + 1 < len(seq):
                        sp_next = emit_scores(b, *seq[i + 1][:2])
                    e_sb = emit_exp(b, qj, ki, sp)
                    emit_attnv(b, qj, ki, nk, e_sb, o_ps)
                    if ki == nk - 1:
                        emit_norm(b, qj, o_ps)
                        for sc4 in range(4):
                            opq.append((b, qj, sc4))
                    elif pending[0] is not None:
                        fn, pending[0] = pending[0], None
                        fn()
                    elif opq:
                        outproj_item(*opq.pop(0))
                    # exp on ACT (~0.93us avg) outpaces the group's PE work
                    # (~0.65us): drain the filler queue with the difference
                    fill_debt[0] += 420
                    while (pending[0] is None and fillers
                           and fill_debt[0] >= fillers[0][0]):
                        cost, fn = fillers.pop(0)
                        fill_debt[0] -= cost
                        pending[0] = fn()

            # ---------------- program ----------------
            pe_warmup(18)
            alloc_qkv(0)
            # batch-0 projections: V first (j 0..3) so transposes interleave,
            # Q in j 3..0 (attention consumes q-tile 3 first), K in j 0..3
            p0 = [(2, 0), (2, 1)]
            p0 += [("t", 0), ("t", 1), (2, 2), ("t", 2), ("t", 3), (2, 3)]
            p0 += [(0, 3), ("t", 4), ("t", 5), (0, 2), ("t", 6), ("t", 7)]
            p0 += [(0, 1), ("t", 8), ("t", 9), (0, 0), ("t", 10), ("t", 11)]
            p0 += [(1, 0), ("t", 12), ("t", 13), (1, 1), ("t", 14), ("t", 15)]
            p0 += [(1, 2), (1, 3)]
            for w_idx, j in p0:
                if w_idx == "t":
                    v_transpose(0, j)
                else:
                    proj_packet(0, w_idx, j)
                    pe_warmup_big(1)

            alloc_qkv(1)
            # filler order: V packets feed transposes; Q packets in j 3..0
            # (b1 attention reads q-tile 3 first), K packets in j 0..3.
            # Packets split in two 4-matmul halves; half A returns half B,
            # which MUST be the next queue-driven small-ring consumer.
            def make_pkt(w, j, b=1):
                def half_a():
                    pps = proj_packet(b, w, j, half=0)
                    return lambda: proj_packet(b, w, j, half=(1, pps))
                return half_a

            fillers = []
            def tr1(s):
                return (250, lambda ss=s: v_transpose(1, ss) and None)

            # batch-1 order: [Q3, V0, tr0-3, K0] unblock its first groups
            # (flushed at the bridge); then V/tr/K waves sized to the groups
            # that consume them, Q j2/j1/j0 last -- all due-date forced
            fillers.append((1700, make_pkt(0, 3)))
            fillers.append((1700, make_pkt(2, 0)))
            fillers += [tr1(0), tr1(1), tr1(2), tr1(3)]
            fillers.append((1700, make_pkt(1, 0)))
            fillers.append((1700, make_pkt(2, 1)))
            fillers += [tr1(4), tr1(5), tr1(6), tr1(7)]
            fillers.append((1700, make_pkt(1, 1)))
            fillers.append((1700, make_pkt(2, 2)))
            fillers += [tr1(8), tr1(9), tr1(10), tr1(11)]
            fillers.append((1700, make_pkt(1, 2)))
            fillers.append((1700, make_pkt(2, 3)))
            fillers += [tr1(12), tr1(13), tr1(14), tr1(15)]
            fillers.append((1700, make_pkt(1, 3)))
            fillers.append((1700, make_pkt(0, 2)))
            fillers.append((1700, make_pkt(0, 1)))
            fillers.append((1700, make_pkt(0, 0)))

            debt = [0]
            opq = []
            pending = [None]
            nf = len(fillers)
            attention(0, fillers, debt, opq, pending,
                      due={28: nf - 2, 32: nf - 4, 36: nf - 7})
            flush_fillers(fillers, pending, keep=nf - 7)
            attention(1, fillers, debt, opq, pending,
                      due={2: 15, 6: 9, 10: 3, 13: 2, 25: 1, 33: 0})
            flush_fillers(fillers, pending)
            while opq:
                outproj_item(*opq.pop(0))

    nc.compile()
    _NC_CACHE[key] = nc
    return nc


def _check_causal(mask: np.ndarray) -> bool:
    m = np.asarray(mask).reshape(mask.shape[-2], mask.shape[-1])
    s = m.shape[0]
    if np.array_equal(m, np.tril(np.ones((s, s), dtype=bool))):
        return True
    if m.all():
        return False
    raise NotImplementedError("only causal or all-true masks are supported")


def kernel(inputs_q, mask, Wq, bq, Wk, bk, Wv, bv, Wo, bo, _trace=False,
           _trace_cores=None):
    import ml_dtypes
    bf16 = ml_dtypes.bfloat16

    inputs_q = np.asarray(inputs_q, dtype=np.float32)
    Wq = np.asarray(Wq, dtype=np.float32).reshape(D, H * HD)
    Wk = np.asarray(Wk, dtype=np.float32).reshape(D, H * HD)
    Wv = np.asarray(Wv, dtype=np.float32).reshape(D, H * HD)
    Wo = np.asarray(Wo, dtype=np.float32).reshape(H * HD, D)
    bq = np.asarray(bq, dtype=np.float32).reshape(H * HD)
    bk = np.asarray(bk, dtype=np.float32).reshape(H * HD)
    bv = np.asarray(bv, dtype=np.float32).reshape(H * HD)
    bo = np.asarray(bo, dtype=np.float32).reshape(D)

    causal = _check_causal(mask)
    with_bias_qkv = bool(bq.any() or bk.any() or bv.any())

    nc = _build_nc(with_bias_qkv, causal)

    xtb = np.ascontiguousarray(inputs_q.transpose(0, 2, 1)).astype(bf16)
    in_maps = []
    for c in range(NCORES):
        f0, f1 = c * HH, (c + 1) * HH
        m = {
            "xt": xtb,
            "wq": np.ascontiguousarray(
                Wq[:, f0:f1].reshape(DC, P, HH).transpose(1, 0, 2)
            ).astype(bf16),
            "wk": np.ascontiguousarray(
                Wk[:, f0:f1].reshape(DC, P, HH).transpose(1, 0, 2)
            ).astype(bf16),
            "wv": np.ascontiguousarray(
                Wv[:, f0:f1].reshape(DC, P, HH).transpose(1, 0, 2)
            ).astype(bf16),
            "wo": np.ascontiguousarray(Wo[f0:f1, :]).astype(bf16),
        }
        if with_bias_qkv:
            m["bqkv"] = np.ascontiguousarray(
                np.stack([bq[f0:f1], bk[f0:f1], bv[f0:f1]])
            )
        in_maps.append(m)

    kwargs = {}
    if _trace:
        kwargs["trace"] = True
        if _trace_cores is not None:
            kwargs["trace_cores"] = _trace_cores
    res = run_bass_kernel_spmd(nc, in_maps, core_ids=list(range(NCORES)), **kwargs)

    acc = np.zeros((B, S, D), dtype=np.float64)
    for c in range(NCORES):
        acc += np.asarray(res.results[c]["out"], dtype=np.float64)
    acc += bo
    out = acc.astype(np.float32)
    if _trace:
        return out, res
    return out
